# revision 10
# baseline (speedup 1.0000x reference)
"""Trainium2 Bass kernel for nn_DehazeBlock:
dilated 3x3 conv (d=2, same-pad) -> BatchNorm2d (training-mode, batch stats)
-> ReLU -> per-4x4-block spatial sort; output concat([a, sorted(a)], channel).

Sharding: data-parallel over batch (16 images -> 8 cores x 2 images).
BN batch stats are all-reduced across the 8 cores inside the kernel.

v3 design (natural-order resident, SBUF residency, 4-quadrant PE tiling):
 - conv bias absorbed by BN mean subtraction -> ignored.
 - sort(relu(affine(x))) == relu(affine(sort(x))) for scale>0, so the 4x4
   block sort runs on RAW conv output; BN affine+ReLU applied in phase 2.
 - Conv rhs APs are CONTIGUOUS 2-row slices (strided matmul rhs gathers
   are ~4x slower on HW - measured); PSUM and the SBUF-resident conv
   stream are in natural row-major order, so the ScalarE PSUM->SBUF copy
   is contiguous. Its accum_out gives the per-channel sum; a second
   AF.Square pass gives the sum of squares.
 - The conv stream stays RESIDENT in SBUF (128 KiB/partition bf16): no
   spill/reload of the a-stream, phase-2 DMA is the 67 MB output only.
 - 4-quadrant PE tiling: two chunks in flight; even chunk on quadrants
   (0,0)/(64,64), odd chunk on (0,64)/(64,0) with the images swapped
   across partition halves (the out-DMA un-swaps by reading the other
   partition half - free).
 - DVE Batcher sort (63 CEs) per chunk: first-layer reads come strided
   from the natural resident (1x mode), everything else contiguous (bf16
   2x). Chunks 0..N_PRE-1 sort during the conv, finals spilled to DRAM;
   the rest after, finals into ordered SBUF buffers.
 - Stats: ScalarE-only chain (Sqrt, exp(-ln(std))) so the DVE sort queue
   never blocks on the AllReduce; the collective overlaps the sort.
 - Phase 2: a-stream relu(scale*x+bias) fully contiguous on ScalarE;
   y-stream the same but with the 4x4 un-permute folded into the dst
   access pattern (strided-dst activation). DMA out from fp32 staging.
"""

import sys

import numpy as np
import ml_dtypes

for _p in ("/opt/trn_rl_repo",):
    if _p not in sys.path:
        sys.path.insert(0, _p)

import concourse.bacc as bacc
import concourse.mybir as mybir
import concourse.tile as tile
from concourse.bass_utils import run_bass_kernel_spmd

NCORES = 8
NB = 2                 # images per core
C = 64                 # channels
H = W = 256
RS = 4                 # block (ranking) size
S = 8                  # block rows per chunk
NCHUNK = (H // RS) // S   # 8 chunks of 32 rows
LANES = RS * RS        # 16 positions within a 4x4 block
BLK = S * (W // RS)    # 512 blocks per chunk (= elements per lane)
CHW = LANES * BLK      # 8192 elements per chunk per partition
CROWS = RS * S         # 32 natural rows per chunk
BROWS = 36             # band rows (32 out rows + 2 halo each side)
BW = 264               # band width allocated (260 used: 2+256+2 pad)
NTOT = NCORES * NB * H * W
EPS = 1e-5

NQUAD = 4              # 4 = two chunks in flight on 4 PE quadrants
N_PRE = 3              # chunks sorted during conv (spilled to DRAM)

F32 = mybir.dt.float32
BF16 = mybir.dt.bfloat16
AF = mybir.ActivationFunctionType
ALU = mybir.AluOpType


# ---------------------------------------------------------------- sort plan
def _batcher_pairs(n=16):
    pairs = []

    def merge(lo, n2, r):
        step = r * 2
        if step < n2:
            merge(lo, n2, step)
            merge(lo + r, n2, step)
            for i in range(lo + r, lo + n2 - r, step):
                pairs.append((i, i + r))
        else:
            pairs.append((lo, lo + r))

    def sort_range(lo, hi):
        if hi - lo >= 1:
            mid = lo + (hi - lo) // 2
            sort_range(lo, mid)
            sort_range(mid + 1, hi)
            merge(lo, hi - lo + 1, 1)

    sort_range(0, n - 1)
    return pairs


def _plan_sort(pairs, n=16):
    """Register-renamed CE plan with direct final placement.

    Inputs start in read-only conv lanes ('c', lane).  Each CE writes min
    to position u and max to position v; a write whose position is never
    touched again goes straight to ('f', rank), else to a scratch slot.
    Returns (steps, nslot); steps = (src_u, src_v, dst_u, dst_v).
    """
    last_touch = {}
    for i, (u, v) in enumerate(pairs):
        last_touch[u] = i
        last_touch[v] = i
    pos = {l: ("c", l) for l in range(n)}
    free, nslot, steps = [], 0, []
    for i, (u, v) in enumerate(pairs):
        su, sv = pos[u], pos[v]
        dst = []
        for p in (u, v):
            if last_touch[p] == i:
                dst.append(("f", p))
            elif free:
                dst.append(("s", free.pop()))
            else:
                dst.append(("s", nslot))
                nslot += 1
        du, dv = dst
        steps.append((su, sv, du, dv))
        for old in (su, sv):
            if old[0] == "s":
                free.append(old[1])
        pos[u], pos[v] = du, dv
    return steps, nslot


def _net60_pairs():
    """Green's 60-comparator 16-sorter (verified by 0/1 principle below)."""
    L = []
    L += [(0, 1), (2, 3), (4, 5), (6, 7), (8, 9), (10, 11), (12, 13),
          (14, 15)]
    L += [(0, 2), (1, 3), (4, 6), (5, 7), (8, 10), (9, 11), (12, 14),
          (13, 15)]
    L += [(0, 4), (1, 5), (2, 6), (3, 7), (8, 12), (9, 13), (10, 14),
          (11, 15)]
    L += [(0, 8), (1, 9), (2, 10), (3, 11), (4, 12), (5, 13), (6, 14),
          (7, 15)]
    L += [(5, 10), (6, 9), (3, 12), (13, 14), (1, 2), (4, 8), (7, 11)]
    L += [(1, 4), (7, 13), (2, 8), (11, 14)]
    L += [(2, 4), (5, 6), (9, 10), (11, 13), (3, 8), (7, 12)]
    L += [(6, 8), (10, 12), (3, 5), (7, 9)]
    L += [(3, 4), (5, 6), (7, 8), (9, 10), (11, 12)]
    L += [(6, 7), (8, 9)]
    return L


def _check_net(pairs, n=16):
    v = ((np.arange(1 << n)[:, None] >> np.arange(n)[None, :]) & 1)
    v = v.astype(np.int8)
    for (a, b) in pairs:
        lo = np.minimum(v[:, a], v[:, b])
        hi = np.maximum(v[:, a], v[:, b])
        v[:, a] = lo
        v[:, b] = hi
    assert (np.diff(v.astype(int), axis=1) >= 0).all(), "bad network"


_PAIRS = _net60_pairs()
_check_net(_PAIRS)
_STEPS, _NSLOT = _plan_sort(_PAIRS)


def _verify_plan():
    rng = np.random.default_rng(0)
    x = rng.standard_normal(LANES)
    mem = {"c": list(x), "s": [0.0] * 32, "f": [0.0] * LANES}
    for su, sv, du, dv in _STEPS:
        a, b = mem[su[0]][su[1]], mem[sv[0]][sv[1]]
        mem[du[0]][du[1]] = min(a, b)
        mem[dv[0]][dv[1]] = max(a, b)
    assert np.allclose(mem["f"], np.sort(x)), "sort plan broken"


_verify_plan()


def _emit_sort(nc, src_of_lane, final_of_rank, slot_of):
    """Emit the CE network on VectorE. All APs are [128, S, W//RS] views."""
    def ap(p):
        if p[0] == "c":
            return src_of_lane(p[1])
        if p[0] == "f":
            return final_of_rank(p[1])
        return slot_of(p[1])

    for su, sv, du, dv in _STEPS:
        a, b = ap(su), ap(sv)
        nc.vector.tensor_tensor(ap(du), a, b, op=ALU.min)
        nc.vector.tensor_tensor(ap(dv), a, b, op=ALU.max)


# ------------------------------------------------------------------- kernel
def _conv_group(nc, band_pool, psum_pool, x, wsb3, resident,
                stats_sum, stats_sq, junk, chunks):
    """Band loads + conv matmuls + psum->resident copies + square pass for
    a group of chunks (2 for NQUAD=4, 1 for NQUAD=2)."""
    bands = []
    for ch in chunks:
        band = band_pool.tile([128, BROWS * BW], BF16, name="band")
        band3 = band.rearrange("p (r w) -> p r w", w=BW)
        r0 = RS * S * ch - 2
        t0 = 2 if ch == 0 else 0
        t1 = 34 if ch == NCHUNK - 1 else 36
        if ch == 0:
            nc.vector.memset(band3[:, 0:2, 0:260], 0.0)
        if ch == NCHUNK - 1:
            nc.vector.memset(band3[:, 34:36, 0:260], 0.0)
        nc.vector.memset(band3[:, t0:t1, 0:2], 0.0)
        nc.vector.memset(band3[:, t0:t1, 258:260], 0.0)
        for m in range(NB):
            nc.sync.dma_start(
                band3[C * m:C * (m + 1), t0:t1, 2:258],
                x[m, :, r0 + t0:r0 + t1, :],
            )
        bands.append(band3)

    # psum tile = 4 natural rows [128, 1024]; 2 matmuls (2 rows) per tap/img
    for h in range(S):
        ps = [psum_pool.tile([128, 2 * BLK], F32, name=f"ps{ci}")
              for ci in range(len(chunks))]
        for t in range(9):
            ky, kx = divmod(t, 3)
            for q in range(2):
                y0 = 4 * h + 2 * q
                for ci, ch in enumerate(chunks):
                    swapped = (NQUAD == 4) and (ch % 2 == 1)
                    out_ap = ps[ci][:, q * BLK:(q + 1) * BLK]
                    for m in range(NB):
                        kp = slice(C * m, C * (m + 1))
                        o0 = C * (1 - m) if swapped else C * m
                        rhs = bands[ci][kp, y0 + 2 * ky:y0 + 2 * ky + 2,
                                        2 * kx:2 * kx + 256]
                        nc.tensor.matmul(
                            out_ap[o0:o0 + C, :], wsb3[kp, t, :], rhs,
                            start=(t == 0), stop=(t == 8),
                            tile_position=(C * m, o0),
                        )
        for ci, ch in enumerate(chunks):
            col = ch * S + h
            dst = resident[:, ch * CHW + h * 2 * BLK:
                           ch * CHW + (h + 1) * 2 * BLK]
            nc.scalar.activation(dst, ps[ci], AF.Copy,
                                 accum_out=stats_sum[:, col:col + 1])
    # sum-of-squares from the resident copy (keeps PSUM recycling fast)
    for ci, ch in enumerate(chunks):
        for k in range(4):
            seg = resident[:, ch * CHW + k * 4 * BLK:
                           ch * CHW + (k + 1) * 4 * BLK]
            nc.scalar.activation(junk, seg, AF.Square,
                                 accum_out=stats_sq[:, ch * 4 + k:
                                                    ch * 4 + k + 1])


def _emit_stats(nc, fp, stats_sum, stats_sq, cc_in, cc_out, gamma, beta,
                scale128, bias128):
    ssum = fp.tile([128, 2], F32)
    jk = fp.tile([128, NCHUNK * S], F32)
    nc.scalar.activation(jk, stats_sum, AF.Copy, accum_out=ssum[:, 0:1])
    nc.scalar.activation(jk[:, 0:NCHUNK * 4], stats_sq, AF.Copy,
                         accum_out=ssum[:, 1:2])
    tmp = fp.tile([C, 2], F32)
    nc.sync.dma_start(tmp, ssum[C:2 * C, :])
    comb = fp.tile([C, 2], F32)
    nc.vector.tensor_add(comb, ssum[0:C, :], tmp)
    nc.sync.dma_start(cc_in, comb)
    nc.gpsimd.collective_compute(
        "AllReduce", ALU.add,
        replica_groups=[list(range(NCORES))],
        ins=[cc_in.opt()], outs=[cc_out.opt()],
    )
    gst = fp.tile([C, 2], F32)
    nc.sync.dma_start(gst, cc_out)

    gam = fp.tile([C, 1], F32)
    bet = fp.tile([C, 1], F32)
    nc.sync.dma_start(gam, gamma)
    nc.sync.dma_start(bet, beta)
    # ScalarE-only scale/bias chain (no DVE op may block the sort queue)
    mean = fp.tile([C, 1], F32)
    nc.scalar.mul(mean, gst[:, 0:1], 1.0 / NTOT)
    epsc = fp.tile([C, 1], F32)
    nc.vector.memset(epsc, EPS)
    ex2p = fp.tile([C, 1], F32)
    nc.scalar.activation(ex2p, gst[:, 1:2], AF.Identity,
                         bias=epsc[:, 0:1], scale=1.0 / NTOT)
    msq = fp.tile([C, 1], F32)
    nc.scalar.square(msq, mean)
    std = fp.tile([C, 1], F32)
    nc.scalar.activation(std, msq, AF.Sqrt, bias=ex2p[:, 0:1], scale=-1.0)
    lns = fp.tile([C, 1], F32)
    nc.scalar.activation(lns, std, AF.Ln)
    rstd = fp.tile([C, 1], F32)
    nc.scalar.activation(rstd, lns, AF.Exp, scale=-1.0)
    sc64 = fp.tile([C, 1], F32)
    nc.scalar.mul(sc64, rstd, gam[:, 0:1])
    t1v = fp.tile([C, 1], F32)
    nc.scalar.mul(t1v, mean, sc64[:, 0:1])
    nb64 = fp.tile([C, 1], F32)
    nc.scalar.activation(nb64, t1v, AF.Identity, bias=bet[:, 0:1], scale=-1.0)
    nc.sync.dma_start(scale128[0:C, :], sc64)
    nc.sync.dma_start(scale128[C:2 * C, :], sc64)
    nc.sync.dma_start(bias128[0:C, :], nb64)
    nc.sync.dma_start(bias128[C:2 * C, :], nb64)


def _emit_p2_chunk(nc, ch, ysrc, resident, stag_pool, scale128, bias128,
                   out):
    """Affine+ReLU (ScalarE) + DMA out for one chunk.

    a-stream: contiguous src (natural resident) -> contiguous stag.
    y-stream: lane-major sorted src -> stag with the 4x4 un-permute folded
    into the dst access pattern.
    Odd chunks have images swapped across partition halves; the out-DMA
    un-swaps by reading the opposite partition half.
    """
    swapped = (NQUAD == 4) and (ch % 2 == 1)
    a_src = resident[:, ch * CHW:(ch + 1) * CHW]
    for c_off, src in ((0, a_src), (C, ysrc)):
        for hh in range(S):                # 4 natural rows per stag tile
            stag = stag_pool.tile([128, 2 * BLK], F32, name="stag")
            nc.scalar.activation(
                stag, src[:, hh * 2 * BLK:(hh + 1) * 2 * BLK], AF.Relu,
                bias=bias128[:, 0:1], scale=scale128[:, 0:1])
            r0 = RS * S * ch + RS * hh
            for m in range(NB):
                p0 = C * (1 - m) if swapped else C * m
                eng = nc.gpsimd if (hh + m) % 2 == 0 else nc.sync
                eng.dma_start(
                    out[m, c_off:c_off + C, r0:r0 + RS, :],
                    stag[p0:p0 + C, :].rearrange("p (r q) -> p r q", q=W),
                )


def _body(tc, nc, x, wT, gamma, beta, out):
    with tc.tile_pool(name="dram", bufs=1, space="DRAM") as dpool, \
         tc.tile_pool(name="persist", bufs=1) as pp, \
         tc.tile_pool(name="fin", bufs=1) as fp:
        spill_d = dpool.tile([128, N_PRE * CHW], BF16)
        cc_in = dpool.tile([C, 2], F32)
        cc_out = dpool.tile([C, 2], F32, addr_space="Shared")

        resident = pp.tile([128, NCHUNK * CHW], BF16)
        wsb = pp.tile([128, 9 * C], BF16)
        stats_sum = pp.tile([128, NCHUNK * S], F32)
        stats_sq = pp.tile([128, NCHUNK * 4], F32)
        scale128 = pp.tile([128, 1], F32)
        bias128 = pp.tile([128, 1], F32)
        scr = pp.tile([128, _NSLOT * BLK], BF16)

        def slot_of(i):
            return scr[:, i * BLK:(i + 1) * BLK].rearrange(
                "p (h w) -> p h w", w=W // RS)

        nc.sync.dma_start(wsb[0:C, :], wT)
        nc.sync.dma_start(wsb[C:2 * C, :], wT)
        wsb3 = wsb.rearrange("p (t co) -> p t co", co=C)

        res4 = resident.rearrange("p (cc r w) -> p cc r w", cc=NCHUNK, w=W)

        def nat_lane(ch, l):
            """Strided natural view of lane l of chunk ch: [p, S, W//RS]."""
            ii, jj = divmod(l, RS)
            return res4[:, ch, ii:ii + 4 * (S - 1) + 1:4,
                        jj:jj + 4 * (W // RS - 1) + 1:4]

        # ---------------- phase 1: conv + stats + presort ----------------
        if NQUAD == 4:
            groups = [(2 * p, 2 * p + 1) for p in range(NCHUNK // 2)]
        else:
            groups = [(ch,) for ch in range(NCHUNK)]

        with tc.tile_pool(name="band", bufs=2) as band_pool, \
             tc.tile_pool(name="psum", bufs=2, space="PSUM") as psum_pool, \
             tc.tile_pool(name="pre", bufs=1) as pre_pool, \
             tc.tile_pool(name="junkp", bufs=1) as junk_pool:
            junk = junk_pool.tile([128, 4 * BLK], BF16)
            pre_emitted = 0

            def nat_rank(buf, r):
                ii, jj = divmod(r, RS)
                b3 = buf.rearrange("p (r w) -> p r w", w=W)
                return b3[:, ii:ii + 4 * (S - 1) + 1:4,
                          jj:jj + 4 * (W // RS - 1) + 1:4]

            for grp in groups:
                _conv_group(nc, band_pool, psum_pool, x, wsb3,
                            resident, stats_sum, stats_sq, junk, grp)
                done = grp[-1] + 1
                while pre_emitted < min(N_PRE, done):
                    ch = pre_emitted
                    pbuf = pre_pool.tile([128, CHW], BF16, name="pre")
                    _emit_sort(
                        nc,
                        lambda l, c=ch: nat_lane(c, l),
                        lambda r, t=pbuf: nat_rank(t, r),
                        slot_of,
                    )
                    nc.gpsimd.dma_start(
                        spill_d[:, ch * CHW:(ch + 1) * CHW], pbuf)
                    pre_emitted += 1

            # stats + collective (ScalarE chain; overlaps the sort)
            _emit_stats(nc, fp, stats_sum, stats_sq, cc_in, cc_out,
                        gamma, beta, scale128, bias128)

        # ---------------- phase 2: sort rest + affine + out ----------------
        with tc.tile_pool(name="sb2", bufs=2) as sb2, \
             tc.tile_pool(name="rl2", bufs=1) as rl2, \
             tc.tile_pool(name="stag", bufs=2) as stag_pool:
            def nat_rank(buf, r):
                ii, jj = divmod(r, RS)
                b3 = buf.rearrange("p (r w) -> p r w", w=W)
                return b3[:, ii:ii + 4 * (S - 1) + 1:4,
                          jj:jj + 4 * (W // RS - 1) + 1:4]

            post = list(range(N_PRE, NCHUNK))
            ysrcs = {}
            for ch in post:
                buf = sb2.tile([128, CHW], BF16, name="sorted")
                _emit_sort(
                    nc,
                    lambda l, c=ch: nat_lane(c, l),
                    lambda r, t=buf: nat_rank(t, r),
                    slot_of,
                )
                ysrcs[ch] = buf

            p2_order = post[:2] + list(range(N_PRE)) + post[2:]
            for pc in p2_order:
                if pc < N_PRE:
                    rbuf = rl2.tile([128, CHW], BF16, name="rl")
                    nc.sync.dma_start(
                        rbuf, spill_d[:, pc * CHW:(pc + 1) * CHW])
                    ysrc = rbuf
                else:
                    ysrc = ysrcs[pc]
                _emit_p2_chunk(nc, pc, ysrc, resident, stag_pool,
                               scale128, bias128, out)


_PROGRAM = None


def _get_program():
    global _PROGRAM
    if _PROGRAM is not None:
        return _PROGRAM
    nc = bacc.Bacc("TRN2", debug=False, enable_asserts=False,
                   target_bir_lowering=False, num_devices=NCORES)
    x = nc.dram_tensor("x", [NB, C, H, W], BF16, kind="ExternalInput").ap()
    wT = nc.dram_tensor("wT", [C, 9 * C], BF16, kind="ExternalInput").ap()
    gamma = nc.dram_tensor("gamma", [C, 1], F32, kind="ExternalInput").ap()
    beta = nc.dram_tensor("beta", [C, 1], F32, kind="ExternalInput").ap()
    out = nc.dram_tensor("out", [NB, 2 * C, H, W], F32,
                         kind="ExternalOutput").ap()
    with tile.TileContext(nc) as tc:
        _body(tc, nc, x, wT, gamma, beta, out)
    nc.compile()
    _PROGRAM = nc
    return nc


def _in_maps(x, conv_w, gamma, beta):
    bf = ml_dtypes.bfloat16
    xq = np.ascontiguousarray(np.asarray(x, np.float32)).astype(bf)
    wTm = np.ascontiguousarray(
        np.asarray(conv_w, np.float32).transpose(1, 2, 3, 0)
    ).reshape(C, 9 * C).astype(bf)
    g = np.ascontiguousarray(np.asarray(gamma, np.float32).reshape(C, 1))
    b = np.ascontiguousarray(np.asarray(beta, np.float32).reshape(C, 1))
    return [
        {"x": xq[NB * k:NB * (k + 1)], "wT": wTm, "gamma": g, "beta": b}
        for k in range(NCORES)
    ]


def run(x, conv_w, conv_b, gamma, beta, **spmd_kwargs):
    nc = _get_program()
    res = run_bass_kernel_spmd(
        nc, _in_maps(x, conv_w, gamma, beta),
        core_ids=list(range(NCORES)), **spmd_kwargs)
    full = np.concatenate(
        [res.results[k]["out"] for k in range(NCORES)], axis=0)
    return full, res


def kernel(x, conv_w, conv_b, gamma, beta):
    full, _ = run(x, conv_w, conv_b, gamma, beta)
    return full


# revision 13
# speedup vs baseline: 1.0407x; 1.0407x over previous
"""Trainium2 Bass kernel for nn_DehazeBlock:
dilated 3x3 conv (d=2, same-pad) -> BatchNorm2d (training-mode, batch stats)
-> ReLU -> per-4x4-block spatial sort; output concat([a, sorted(a)], channel).

Sharding: data-parallel over batch (16 images -> 8 cores x 2 images).
BN batch stats are all-reduced across the 8 cores inside the kernel.

v3 design (natural-order resident, SBUF residency, 4-quadrant PE tiling):
 - conv bias absorbed by BN mean subtraction -> ignored.
 - sort(relu(affine(x))) == relu(affine(sort(x))) for scale>0, so the 4x4
   block sort runs on RAW conv output; BN affine+ReLU applied in phase 2.
 - Conv rhs APs are CONTIGUOUS 2-row slices (strided matmul rhs gathers
   are ~4x slower on HW - measured); PSUM and the SBUF-resident conv
   stream are in natural row-major order, so the ScalarE PSUM->SBUF copy
   is contiguous. Its accum_out gives the per-channel sum; a second
   AF.Square pass gives the sum of squares.
 - The conv stream stays RESIDENT in SBUF (128 KiB/partition bf16): no
   spill/reload of the a-stream, phase-2 DMA is the 67 MB output only.
 - 4-quadrant PE tiling: two chunks in flight; even chunk on quadrants
   (0,0)/(64,64), odd chunk on (0,64)/(64,0) with the images swapped
   across partition halves (the out-DMA un-swaps by reading the other
   partition half - free).
 - DVE Batcher sort (63 CEs) per chunk: first-layer reads come strided
   from the natural resident (1x mode), everything else contiguous (bf16
   2x). Chunks 0..N_PRE-1 sort during the conv, finals spilled to DRAM;
   the rest after, finals into ordered SBUF buffers.
 - Stats: ScalarE-only chain (Sqrt, exp(-ln(std))) so the DVE sort queue
   never blocks on the AllReduce; the collective overlaps the sort.
 - Phase 2: a-stream relu(scale*x+bias) fully contiguous on ScalarE;
   y-stream the same but with the 4x4 un-permute folded into the dst
   access pattern (strided-dst activation). DMA out from fp32 staging.
"""

import sys

import numpy as np
import ml_dtypes

for _p in ("/opt/trn_rl_repo",):
    if _p not in sys.path:
        sys.path.insert(0, _p)

import concourse.bacc as bacc
import concourse.mybir as mybir
import concourse.tile as tile
from concourse.bass_utils import run_bass_kernel_spmd

NCORES = 8
NB = 2                 # images per core
C = 64                 # channels
H = W = 256
RS = 4                 # block (ranking) size
S = 8                  # block rows per chunk
NCHUNK = (H // RS) // S   # 8 chunks of 32 rows
LANES = RS * RS        # 16 positions within a 4x4 block
BLK = S * (W // RS)    # 512 blocks per chunk (= elements per lane)
CHW = LANES * BLK      # 8192 elements per chunk per partition
CROWS = RS * S         # 32 natural rows per chunk
BROWS = 36             # band rows (32 out rows + 2 halo each side)
BW = 264               # band width allocated (260 used: 2+256+2 pad)
NTOT = NCORES * NB * H * W
EPS = 1e-5

NQUAD = 4              # 4 = two chunks in flight on 4 PE quadrants
N_PRE = 3              # chunks sorted during conv (spilled to DRAM)

F32 = mybir.dt.float32
BF16 = mybir.dt.bfloat16
AF = mybir.ActivationFunctionType
ALU = mybir.AluOpType


# ---------------------------------------------------------------- sort plan
def _batcher_pairs(n=16):
    pairs = []

    def merge(lo, n2, r):
        step = r * 2
        if step < n2:
            merge(lo, n2, step)
            merge(lo + r, n2, step)
            for i in range(lo + r, lo + n2 - r, step):
                pairs.append((i, i + r))
        else:
            pairs.append((lo, lo + r))

    def sort_range(lo, hi):
        if hi - lo >= 1:
            mid = lo + (hi - lo) // 2
            sort_range(lo, mid)
            sort_range(mid + 1, hi)
            merge(lo, hi - lo + 1, 1)

    sort_range(0, n - 1)
    return pairs


def _plan_sort(pairs, n=16):
    """Register-renamed CE plan with direct final placement.

    Inputs start in read-only conv lanes ('c', lane).  Each CE writes min
    to position u and max to position v; a write whose position is never
    touched again goes straight to ('f', rank), else to a scratch slot.
    Returns (steps, nslot); steps = (src_u, src_v, dst_u, dst_v).
    """
    last_touch = {}
    for i, (u, v) in enumerate(pairs):
        last_touch[u] = i
        last_touch[v] = i
    pos = {l: ("c", l) for l in range(n)}
    free, nslot, steps = [], 0, []
    for i, (u, v) in enumerate(pairs):
        su, sv = pos[u], pos[v]
        dst = []
        for p in (u, v):
            if last_touch[p] == i:
                dst.append(("f", p))
            elif free:
                dst.append(("s", free.pop()))
            else:
                dst.append(("s", nslot))
                nslot += 1
        du, dv = dst
        steps.append((su, sv, du, dv))
        for old in (su, sv):
            if old[0] == "s":
                free.append(old[1])
        pos[u], pos[v] = du, dv
    return steps, nslot


def _net60_pairs():
    """Green's 60-comparator 16-sorter (verified by 0/1 principle below)."""
    L = []
    L += [(0, 1), (2, 3), (4, 5), (6, 7), (8, 9), (10, 11), (12, 13),
          (14, 15)]
    L += [(0, 2), (1, 3), (4, 6), (5, 7), (8, 10), (9, 11), (12, 14),
          (13, 15)]
    L += [(0, 4), (1, 5), (2, 6), (3, 7), (8, 12), (9, 13), (10, 14),
          (11, 15)]
    L += [(0, 8), (1, 9), (2, 10), (3, 11), (4, 12), (5, 13), (6, 14),
          (7, 15)]
    L += [(5, 10), (6, 9), (3, 12), (13, 14), (1, 2), (4, 8), (7, 11)]
    L += [(1, 4), (7, 13), (2, 8), (11, 14)]
    L += [(2, 4), (5, 6), (9, 10), (11, 13), (3, 8), (7, 12)]
    L += [(6, 8), (10, 12), (3, 5), (7, 9)]
    L += [(3, 4), (5, 6), (7, 8), (9, 10), (11, 12)]
    L += [(6, 7), (8, 9)]
    return L


def _check_net(pairs, n=16):
    v = ((np.arange(1 << n)[:, None] >> np.arange(n)[None, :]) & 1)
    v = v.astype(np.int8)
    for (a, b) in pairs:
        lo = np.minimum(v[:, a], v[:, b])
        hi = np.maximum(v[:, a], v[:, b])
        v[:, a] = lo
        v[:, b] = hi
    assert (np.diff(v.astype(int), axis=1) >= 0).all(), "bad network"


_PAIRS = _net60_pairs()
_check_net(_PAIRS)
_STEPS, _NSLOT = _plan_sort(_PAIRS)


def _verify_plan():
    rng = np.random.default_rng(0)
    x = rng.standard_normal(LANES)
    mem = {"c": list(x), "s": [0.0] * 32, "f": [0.0] * LANES}
    for su, sv, du, dv in _STEPS:
        a, b = mem[su[0]][su[1]], mem[sv[0]][sv[1]]
        mem[du[0]][du[1]] = min(a, b)
        mem[dv[0]][dv[1]] = max(a, b)
    assert np.allclose(mem["f"], np.sort(x)), "sort plan broken"


_verify_plan()


def _emit_sort(nc, src_of_lane, final_of_rank, slot_of):
    """Emit the CE network on VectorE. All APs are [128, S, W//RS] views."""
    def ap(p):
        if p[0] == "c":
            return src_of_lane(p[1])
        if p[0] == "f":
            return final_of_rank(p[1])
        return slot_of(p[1])

    for su, sv, du, dv in _STEPS:
        a, b = ap(su), ap(sv)
        nc.vector.tensor_tensor(ap(du), a, b, op=ALU.min)
        nc.vector.tensor_tensor(ap(dv), a, b, op=ALU.max)


# ------------------------------------------------------------------- kernel
def _conv_group(nc, band_pool, psum_pool, x, wsb3, resident,
                stats_sum, stats_sq, junk, chunks):
    """Band loads + conv matmuls + psum->resident copies + square pass for
    a group of chunks (2 for NQUAD=4, 1 for NQUAD=2)."""
    bands = []
    for ch in chunks:
        band = band_pool.tile([128, BROWS * BW], BF16, name="band")
        band3 = band.rearrange("p (r w) -> p r w", w=BW)
        r0 = RS * S * ch - 2
        t0 = 2 if ch == 0 else 0
        t1 = 34 if ch == NCHUNK - 1 else 36
        if ch == 0:
            nc.vector.memset(band3[:, 0:2, 0:260], 0.0)
        if ch == NCHUNK - 1:
            nc.vector.memset(band3[:, 34:36, 0:260], 0.0)
        nc.vector.memset(band3[:, t0:t1, 0:2], 0.0)
        nc.vector.memset(band3[:, t0:t1, 258:260], 0.0)
        for m in range(NB):
            nc.sync.dma_start(
                band3[C * m:C * (m + 1), t0:t1, 2:258],
                x[m, :, r0 + t0:r0 + t1, :],
            )
        bands.append(band3)

    # psum tile = 4 natural rows [128, 1024]; 2 matmuls (2 rows) per tap/img
    for h in range(S):
        ps = [psum_pool.tile([128, 2 * BLK], F32, name=f"ps{ci}")
              for ci in range(len(chunks))]
        for t in range(9):
            ky, kx = divmod(t, 3)
            for q in range(2):
                y0 = 4 * h + 2 * q
                for ci, ch in enumerate(chunks):
                    swapped = (NQUAD == 4) and (ch % 2 == 1)
                    out_ap = ps[ci][:, q * BLK:(q + 1) * BLK]
                    for m in range(NB):
                        kp = slice(C * m, C * (m + 1))
                        o0 = C * (1 - m) if swapped else C * m
                        rhs = bands[ci][kp, y0 + 2 * ky:y0 + 2 * ky + 2,
                                        2 * kx:2 * kx + 256]
                        nc.tensor.matmul(
                            out_ap[o0:o0 + C, :], wsb3[kp, t, :], rhs,
                            start=(t == 0), stop=(t == 8),
                            tile_position=(C * m, o0),
                        )
        for ci, ch in enumerate(chunks):
            col = ch * S + h
            dst = resident[:, ch * CHW + h * 2 * BLK:
                           ch * CHW + (h + 1) * 2 * BLK]
            nc.scalar.activation(dst, ps[ci], AF.Copy,
                                 accum_out=stats_sum[:, col:col + 1])
        # interleave one sum-of-squares op per h-step so copies never queue
        # behind a block of squares (PSUM recycling stays fast)
        if h % 2 == 1:
            k = h // 2
            for ci, ch in enumerate(chunks):
                seg = resident[:, ch * CHW + k * 4 * BLK:
                               ch * CHW + (k + 1) * 4 * BLK]
                nc.scalar.activation(junk, seg, AF.Square,
                                     accum_out=stats_sq[:, ch * 4 + k:
                                                        ch * 4 + k + 1])


def _emit_stats(nc, fp, stats_sum, stats_sq, cc_in, cc_out, gamma, beta,
                scale128, bias128):
    ssum = fp.tile([128, 2], F32)
    jk = fp.tile([128, NCHUNK * S], F32)
    nc.scalar.activation(jk, stats_sum, AF.Copy, accum_out=ssum[:, 0:1])
    nc.scalar.activation(jk[:, 0:NCHUNK * 4], stats_sq, AF.Copy,
                         accum_out=ssum[:, 1:2])
    tmp = fp.tile([C, 2], F32)
    nc.sync.dma_start(tmp, ssum[C:2 * C, :])
    comb = fp.tile([C, 2], F32)
    nc.vector.tensor_add(comb, ssum[0:C, :], tmp)
    nc.sync.dma_start(cc_in, comb)
    nc.gpsimd.collective_compute(
        "AllReduce", ALU.add,
        replica_groups=[list(range(NCORES))],
        ins=[cc_in.opt()], outs=[cc_out.opt()],
    )
    gst = fp.tile([C, 2], F32)
    nc.sync.dma_start(gst, cc_out)

    gam = fp.tile([C, 1], F32)
    bet = fp.tile([C, 1], F32)
    nc.sync.dma_start(gam, gamma)
    nc.sync.dma_start(bet, beta)
    # ScalarE-only scale/bias chain (no DVE op may block the sort queue)
    mean = fp.tile([C, 1], F32)
    nc.scalar.mul(mean, gst[:, 0:1], 1.0 / NTOT)
    epsc = fp.tile([C, 1], F32)
    nc.vector.memset(epsc, EPS)
    ex2p = fp.tile([C, 1], F32)
    nc.scalar.activation(ex2p, gst[:, 1:2], AF.Identity,
                         bias=epsc[:, 0:1], scale=1.0 / NTOT)
    msq = fp.tile([C, 1], F32)
    nc.scalar.square(msq, mean)
    std = fp.tile([C, 1], F32)
    nc.scalar.activation(std, msq, AF.Sqrt, bias=ex2p[:, 0:1], scale=-1.0)
    lns = fp.tile([C, 1], F32)
    nc.scalar.activation(lns, std, AF.Ln)
    rstd = fp.tile([C, 1], F32)
    nc.scalar.activation(rstd, lns, AF.Exp, scale=-1.0)
    sc64 = fp.tile([C, 1], F32)
    nc.scalar.mul(sc64, rstd, gam[:, 0:1])
    t1v = fp.tile([C, 1], F32)
    nc.scalar.mul(t1v, mean, sc64[:, 0:1])
    nb64 = fp.tile([C, 1], F32)
    nc.scalar.activation(nb64, t1v, AF.Identity, bias=bet[:, 0:1], scale=-1.0)
    nc.sync.dma_start(scale128[0:C, :], sc64)
    nc.sync.dma_start(scale128[C:2 * C, :], sc64)
    nc.sync.dma_start(bias128[0:C, :], nb64)
    nc.sync.dma_start(bias128[C:2 * C, :], nb64)


def _emit_p2_chunk(nc, ch, ysrc, resident, stag_pool, scale128, bias128,
                   out):
    """Affine+ReLU (ScalarE) + DMA out for one chunk.

    a-stream: contiguous src (natural resident) -> contiguous stag.
    y-stream: lane-major sorted src -> stag with the 4x4 un-permute folded
    into the dst access pattern.
    Odd chunks have images swapped across partition halves; the out-DMA
    un-swaps by reading the opposite partition half.
    """
    swapped = (NQUAD == 4) and (ch % 2 == 1)
    a_src = resident[:, ch * CHW:(ch + 1) * CHW]
    for c_off, src in ((0, a_src), (C, ysrc)):
        for hh in range(S):                # 4 natural rows per stag tile
            stag = stag_pool.tile([128, 2 * BLK], F32, name="stag")
            if c_off == 0:
                nc.scalar.activation(
                    stag, src[:, hh * 2 * BLK:(hh + 1) * 2 * BLK], AF.Relu,
                    bias=bias128[:, 0:1], scale=scale128[:, 0:1])
            else:
                # sorted buf is (rank, hb, wb); iterate (ii, wb, jj) so the
                # dst (natural) gets contiguous 4-element runs and the src
                # carries the stride
                stag4 = stag.rearrange("p (ii wb jj) -> p ii wb jj",
                                       ii=RS, wb=W // RS, jj=RS)
                ysrc4 = src.rearrange("p (ii jj hb wb) -> p ii jj hb wb",
                                      ii=RS, jj=RS, hb=S, wb=W // RS)
                nc.scalar.activation(
                    stag4,
                    ysrc4[:, :, :, hh, :].transpose([0, 1, 3, 2]),
                    AF.Relu,
                    bias=bias128[:, 0:1], scale=scale128[:, 0:1])
            r0 = RS * S * ch + RS * hh
            for m in range(NB):
                p0 = C * (1 - m) if swapped else C * m
                eng = nc.gpsimd if (hh + m) % 2 == 0 else nc.sync
                eng.dma_start(
                    out[m, c_off:c_off + C, r0:r0 + RS, :],
                    stag[p0:p0 + C, :].rearrange("p (r q) -> p r q", q=W),
                )


def _body(tc, nc, x, wT, gamma, beta, out):
    with tc.tile_pool(name="dram", bufs=1, space="DRAM") as dpool, \
         tc.tile_pool(name="persist", bufs=1) as pp, \
         tc.tile_pool(name="fin", bufs=1) as fp:
        spill_d = dpool.tile([128, N_PRE * CHW], BF16)
        cc_in = dpool.tile([C, 2], F32)
        cc_out = dpool.tile([C, 2], F32, addr_space="Shared")

        resident = pp.tile([128, NCHUNK * CHW], BF16)
        wsb = pp.tile([128, 9 * C], BF16)
        stats_sum = pp.tile([128, NCHUNK * S], F32)
        stats_sq = pp.tile([128, NCHUNK * 4], F32)
        scale128 = pp.tile([128, 1], F32)
        bias128 = pp.tile([128, 1], F32)
        scr = pp.tile([128, _NSLOT * BLK], BF16)

        def slot_of(i):
            return scr[:, i * BLK:(i + 1) * BLK].rearrange(
                "p (h w) -> p h w", w=W // RS)

        nc.sync.dma_start(wsb[0:C, :], wT)
        nc.sync.dma_start(wsb[C:2 * C, :], wT)
        wsb3 = wsb.rearrange("p (t co) -> p t co", co=C)

        res4 = resident.rearrange("p (cc r w) -> p cc r w", cc=NCHUNK, w=W)

        def nat_lane(ch, l):
            """Strided natural view of lane l of chunk ch: [p, S, W//RS]."""
            ii, jj = divmod(l, RS)
            return res4[:, ch, ii:ii + 4 * (S - 1) + 1:4,
                        jj:jj + 4 * (W // RS - 1) + 1:4]

        # ---------------- phase 1: conv + stats + presort ----------------
        if NQUAD == 4:
            groups = [(2 * p, 2 * p + 1) for p in range(NCHUNK // 2)]
        else:
            groups = [(ch,) for ch in range(NCHUNK)]

        with tc.tile_pool(name="band", bufs=2) as band_pool, \
             tc.tile_pool(name="psum", bufs=2, space="PSUM") as psum_pool, \
             tc.tile_pool(name="pre", bufs=1) as pre_pool, \
             tc.tile_pool(name="junkp", bufs=1) as junk_pool:
            junk = junk_pool.tile([128, 4 * BLK], BF16)
            pre_emitted = 0

            def rank_slot(buf, r):
                return buf[:, r * BLK:(r + 1) * BLK].rearrange(
                    "p (h w) -> p h w", w=W // RS)

            for grp in groups:
                _conv_group(nc, band_pool, psum_pool, x, wsb3,
                            resident, stats_sum, stats_sq, junk, grp)
                done = grp[-1] + 1
                while pre_emitted < min(N_PRE, done):
                    ch = pre_emitted
                    pbuf = pre_pool.tile([128, CHW], BF16, name="pre")
                    _emit_sort(
                        nc,
                        lambda l, c=ch: nat_lane(c, l),
                        lambda r, t=pbuf: rank_slot(t, r),
                        slot_of,
                    )
                    nc.gpsimd.dma_start(
                        spill_d[:, ch * CHW:(ch + 1) * CHW], pbuf)
                    pre_emitted += 1

            # stats + collective (ScalarE chain; overlaps the sort)
            _emit_stats(nc, fp, stats_sum, stats_sq, cc_in, cc_out,
                        gamma, beta, scale128, bias128)

        # ---------------- phase 2: sort rest + affine + out ----------------
        with tc.tile_pool(name="sb2", bufs=2) as sb2, \
             tc.tile_pool(name="rl2", bufs=1) as rl2, \
             tc.tile_pool(name="stag", bufs=2) as stag_pool:
            def rank_slot(buf, r):
                return buf[:, r * BLK:(r + 1) * BLK].rearrange(
                    "p (h w) -> p h w", w=W // RS)

            post = list(range(N_PRE, NCHUNK))
            ysrcs = {}
            for ch in post:
                buf = sb2.tile([128, CHW], BF16, name="sorted")
                _emit_sort(
                    nc,
                    lambda l, c=ch: nat_lane(c, l),
                    lambda r, t=buf: rank_slot(t, r),
                    slot_of,
                )
                ysrcs[ch] = buf

            p2_order = post[:2] + list(range(N_PRE)) + post[2:]
            for pc in p2_order:
                if pc < N_PRE:
                    rbuf = rl2.tile([128, CHW], BF16, name="rl")
                    nc.sync.dma_start(
                        rbuf, spill_d[:, pc * CHW:(pc + 1) * CHW])
                    ysrc = rbuf
                else:
                    ysrc = ysrcs[pc]
                _emit_p2_chunk(nc, pc, ysrc, resident, stag_pool,
                               scale128, bias128, out)


_PROGRAM = None


def _get_program():
    global _PROGRAM
    if _PROGRAM is not None:
        return _PROGRAM
    nc = bacc.Bacc("TRN2", debug=False, enable_asserts=False,
                   target_bir_lowering=False, num_devices=NCORES)
    x = nc.dram_tensor("x", [NB, C, H, W], BF16, kind="ExternalInput").ap()
    wT = nc.dram_tensor("wT", [C, 9 * C], BF16, kind="ExternalInput").ap()
    gamma = nc.dram_tensor("gamma", [C, 1], F32, kind="ExternalInput").ap()
    beta = nc.dram_tensor("beta", [C, 1], F32, kind="ExternalInput").ap()
    out = nc.dram_tensor("out", [NB, 2 * C, H, W], F32,
                         kind="ExternalOutput").ap()
    with tile.TileContext(nc) as tc:
        _body(tc, nc, x, wT, gamma, beta, out)
    nc.compile()
    _PROGRAM = nc
    return nc


def _in_maps(x, conv_w, gamma, beta):
    bf = ml_dtypes.bfloat16
    xq = np.ascontiguousarray(np.asarray(x, np.float32)).astype(bf)
    wTm = np.ascontiguousarray(
        np.asarray(conv_w, np.float32).transpose(1, 2, 3, 0)
    ).reshape(C, 9 * C).astype(bf)
    g = np.ascontiguousarray(np.asarray(gamma, np.float32).reshape(C, 1))
    b = np.ascontiguousarray(np.asarray(beta, np.float32).reshape(C, 1))
    return [
        {"x": xq[NB * k:NB * (k + 1)], "wT": wTm, "gamma": g, "beta": b}
        for k in range(NCORES)
    ]


def run(x, conv_w, conv_b, gamma, beta, **spmd_kwargs):
    nc = _get_program()
    res = run_bass_kernel_spmd(
        nc, _in_maps(x, conv_w, gamma, beta),
        core_ids=list(range(NCORES)), **spmd_kwargs)
    full = np.concatenate(
        [res.results[k]["out"] for k in range(NCORES)], axis=0)
    return full, res


def kernel(x, conv_w, conv_b, gamma, beta):
    full, _ = run(x, conv_w, conv_b, gamma, beta)
    return full
